# revision 35
# baseline (speedup 1.0000x reference)
"""Trainium2 Bass kernel for a hard-triplet margin-ranking loss.

Sharding: data-parallel over anchor rows, 8 cores x 512 anchors. Rows in the
first half of the batch mine over columns [2048:4096], second half over
[0:2048]; each core computes its 512x2048 slice of the distance matrix.

Host prep (free, outside HW time): cast features to fp16 and tile them
[128, 20*256] (tile t = rows 128t..128t+127). fp16 halves DMA bytes
(memory-bound regime) and unlocks full-rate PE matmuls (1cyc/row vs 4 for
fp32) plus 2x/4x DVE perf modes.

Per core, per group g of 4 row tiles (pipelined):
  1. DMA the group's natural-layout fp16 x.
  2. Row norms: ACT Square+accum per [128,256] tile; per-group stat chain
     inv = 1/(sqrt(sq)+eps) on [128,4] (anchor group scaled by -0.25 so the
     matmul yields pm = -0.25*<xn_i,xn_j>, dist^2 = 2 + 8*pm).
  3. Normalize: DVE tensor_scalar (4x mode, fp16) into xnall.
  4. One XBAR DMA-transpose [128,1024] -> xT2[:, 8g:8g+8, :]; block e=2t+c
     holds (tile t, dim-chunk c) columns. No PE transposes, no PSUM evac.
Then:
  5. Main matmul: stationary operand (anchor block) reused across 4 moving
     chunks -> 8 Ldweights total; fp16, K=256 via 2 PSUM-accumulated chunks;
     pm [128,2048] f32, double buffered.
  6. w = pm + (t_o == t_b): mask built per row tile (DVE tensor_scalar
     is_equal, 4x); added while evacuating PSUM (engine per CFG knob).
     Matched cols land in [0.75,1.25], unmatched in [-0.25,0.25].
  7. Row max/min via pairwise tensor_tensor trees (2x fp16) on DVE.
  8. Epilogue as in the reference; ones-matmul row-sum; host sums 8
     partials / 4096.
"""

import numpy as np

N, D = 4096, 256
HALF = N // 2
NCORES = 8
RPC = N // NCORES   # 512 anchor rows per core
RT = RPC // 128     # 4 anchor row tiles
OT = HALF // 128    # 16 opposite-half tiles
NT = RT + OT        # 20 input tiles
NG = NT // 4        # 5 groups of 4 tiles
MARGIN = 0.3
EPS = 1e-6

# Engine tuning knobs.
#   wadd[r] in {"dve", "act"}: how pm+mask leaves PSUM for row tile r.
CFG = {
    "wadd2": [["dve", "act", "act", "act"], ["dve", "act", "act", "act"]],
    "sq_dve": set(),
}

_CACHE = {}


def _build():
    from contextlib import ExitStack

    import concourse.bacc as bacc
    import concourse.bass as bass
    import concourse.tile as tile
    from concourse import mybir

    def gsl2(g):
        return slice(g * 4 * D, (g + 1) * 4 * D)

    f32 = mybir.dt.float32
    f16 = mybir.dt.float16
    Alu = mybir.AluOpType
    Act = mybir.ActivationFunctionType
    AxX = mybir.AxisListType.X
    ts = bass.ts

    nc = bacc.Bacc(
        "TRN2",
        target_bir_lowering=False,
        debug=False,
        enable_asserts=True,
        num_devices=NCORES,
    )
    xin = nc.dram_tensor("xin", [128, NT * D], f16, kind="ExternalInput").ap()
    tb = nc.dram_tensor("tb", [128, RT], f32, kind="ExternalInput").ap()
    to = nc.dram_tensor("to", [1, HALF], f16, kind="ExternalInput").ap()
    out = nc.dram_tensor("out", [128, RT], f32, kind="ExternalOutput").ap()

    with tile.TileContext(nc) as tc, ExitStack() as ctx:
        const = ctx.enter_context(tc.tile_pool(name="const", bufs=1))
        xpool = ctx.enter_context(tc.tile_pool(name="xpool", bufs=1))
        stat = ctx.enter_context(tc.tile_pool(name="stat", bufs=1))
        scr = ctx.enter_context(tc.tile_pool(name="scr", bufs=4))
        wmask = ctx.enter_context(tc.tile_pool(name="wmask", bufs=2))
        wpool = ctx.enter_context(tc.tile_pool(name="wpool", bufs=1))
        tree = ctx.enter_context(tc.tile_pool(name="tree", bufs=1))
        psm = ctx.enter_context(tc.tile_pool(name="psm", bufs=3, space="PSUM"))
        psn = ctx.enter_context(tc.tile_pool(name="psn", bufs=2, space="PSUM"))

        ones = const.tile([128, 1], f32, tag="ones")
        nc.vector.memset(ones[:], 1.0)
        # Pin the ACT function table (sqrt_and_others: Square/Sqrt/Copy/Relu)
        # once, while DMAs are in flight, to avoid a mid-kernel reload.
        warm = const.tile([1, 1], f32, tag="warm")
        nc.scalar.activation(warm[:], ones[0:1, :], Act.Sqrt)
        b_m6 = const.tile([128, 1], f32, tag="b_m6")
        nc.vector.memset(b_m6[:], -6.0)
        b_2me = const.tile([128, 1], f32, tag="b_2me")
        nc.vector.memset(b_2me[:], 2.0 - EPS)
        b_eps = const.tile([128, 1], f32, tag="b_eps")
        nc.vector.memset(b_eps[:], EPS)


        # Phase-1, transposed space: XBAR-transpose raw fp16 straight from
        # DRAM (one DMA per 4-tile group, no natural-layout copy), square it,
        # partition-sum via a PE ones-matmul (every output partition gets the
        # row norm^2 -> no layout shuffle), sqrt (ACT) + reciprocal (DVE),
        # then scale columns (rows of x) with 2 strided tensor_tensor ops per
        # group. 1/(||x||+eps) ~= 1/||x|| to 6e-8 relative, far below fp16
        # noise, so eps is dropped.
        xT2r = xpool.tile([128, 2 * NT, 128], f16, tag="xT2r")
        x2T = xpool.tile([128, 2 * NT, 128], f16, tag="x2T")
        xT2 = xpool.tile([128, 2 * NT, 128], f16, tag="xT2")
        invw = xpool.tile([128, NT * 128], f16, tag="invw")
        ones16 = const.tile([128, 128], f16, tag="ones16")
        nc.vector.memset(ones16[:], 1.0)

        for g in range(NG):
            nc.sync.dma_start(
                xT2r[:, 8 * g : 8 * (g + 1), :], xin[:, gsl2(g)], transpose=True
            )
        # Targets after the transposes in the SP queue (needed much later).
        to_row = const.tile([1, HALF], f16, tag="to_row")
        nc.sync.dma_start(to_row[:], to[:])
        tob = const.tile([128, HALF], f16, tag="tob")
        nc.gpsimd.partition_broadcast(tob[:], to_row[:])
        tbt = const.tile([128, RT], f32, tag="tbt")
        nc.sync.dma_start(tbt[:], tb[:])

        xrc = xT2r[:].rearrange("p (t c) r -> p c t r", c=2)
        x2c = x2T[:].rearrange("p (t c) r -> p c t r", c=2)
        xfc = xT2[:].rearrange("p (t c) r -> p c t r", c=2)
        ivc = invw[:].rearrange("p (t r) -> p t r", r=128)

        for g in range(NG):
            gs8 = slice(8 * g, 8 * (g + 1))
            if g in CFG["sq_dve"]:
                nc.vector.tensor_tensor(
                    x2T[:, gs8, :], xT2r[:, gs8, :], xT2r[:, gs8, :], op=Alu.mult
                )
            else:
                nc.scalar.activation(x2T[:, gs8, :], xT2r[:, gs8, :], Act.Square)
            nq = psn.tile([128, 512], f32, tag="nq")
            for c in range(2):
                nc.tensor.matmul(
                    nq[:],
                    lhsT=ones16[:],
                    rhs=x2c[:, c, 4 * g : 4 * (g + 1), :],
                    start=(c == 0),
                    stop=(c == 1),
                )
            nrmw = scr.tile([128, 512], f32, tag="nrmw")
            nc.scalar.activation(nrmw[:], nq[:], Act.Sqrt)
            iw = invw[:, 512 * g : 512 * (g + 1)]
            with nc.allow_low_precision(reason="inv scale consumed in fp16"):
                nc.vector.reciprocal(iw, nrmw[:])
            if g == 0:
                nc.vector.tensor_scalar_mul(iw, iw, -0.25)
            for c in range(2):
                nc.vector.tensor_tensor(
                    xfc[:, c, 4 * g : 4 * (g + 1), :],
                    xrc[:, c, 4 * g : 4 * (g + 1), :],
                    ivc[:, 4 * g : 4 * (g + 1), :],
                    op=Alu.mult,
                )

        wall = wpool.tile([128, RT, HALF], f16, tag="wall")
        for r in range(RT):
            nc.vector.tensor_scalar(wall[:, r, :], tob[:], tbt[:, r : r + 1],
                                    None, op0=Alu.is_equal)

        # Chunk view: xTc[p, c, t, r] = dim 128c+p of row 128t+r.
        xTc = xT2[:].rearrange("p (t c) r -> p c t r", c=2)

        # Main matmul in two half-rounds: half hp covers distance columns
        # [1024*hp, 1024*(hp+1)) and needs only opposite groups 1+2hp, 2+2hp,
        # so round 0 starts before the last transposes land. PSUM: 4 live
        # [128,1024] halves. Evacuation: ACT copy -> fp16, then a gpsimd
        # SWDGE DMA accumulates it onto the mask in w (DVE untouched).
        mx = stat.tile([128, RT], f16, tag="mx")
        mn = stat.tile([128, RT], f16, tag="mn")
        hpart = {}
        for p in range(2):
            for h in range(2):
                hp_t = tree.tile([128, RT, 512], f16, tag=f"hp_{p}_{h}")
                hpart[(p, h)] = hp_t
        for hp in range(2):
            for r in range(RT):
                pm = psm.tile([128, 1024], f32, tag="pm")
                for c in range(2):
                    for k in range(2):
                        hk = 2 * hp + k
                        nc.tensor.matmul(
                            pm[:, ts(k, 512)],
                            lhsT=xTc[:, c, r, :],
                            rhs=xTc[:, c, 4 + 4 * hk : 8 + 4 * hk, :],
                            start=(c == 0),
                            stop=(c == 1),
                        )
                wh = wall[:, r, 1024 * hp : 1024 * (hp + 1)]
                if CFG["wadd2"][hp][r] == "dve":
                    nc.vector.tensor_tensor(wh, pm[:], wh, op=Alu.add)
                else:
                    tmp = scr.tile([128, 1024], f16, tag="evac")
                    nc.scalar.copy(tmp[:], pm[:])
                    nc.gpsimd.dma_start(wh, tmp[:], accum_op=Alu.add)
            # Pair-batched per-half trees: set s covers row tiles 2s,2s+1 so
            # reduction starts after two adds instead of four.
            for s in range(2):
                whv = wall[:, 2 * s : 2 * s + 2, 1024 * hp : 1024 * (hp + 1)]
                for p, op in ((0, Alu.max), (1, Alu.min)):
                    l2 = hpart[(p, hp)][:, 2 * s : 2 * s + 2, :]
                    nc.vector.tensor_tensor(
                        l2, whv[:, :, 0:512], whv[:, :, 512:1024], op=op
                    )
                    nc.vector.tensor_tensor(
                        l2[:, :, 0:256], l2[:, :, 0:256], l2[:, :, 256:512], op=op
                    )
                    nc.vector.tensor_tensor(
                        l2[:, :, 0:128], l2[:, :, 0:128], l2[:, :, 128:256], op=op
                    )
        for p, op in ((0, Alu.max), (1, Alu.min)):
            l4 = tree.tile([128, RT, 128], f16, tag=f"l4_{p}_f")
            nc.vector.tensor_tensor(
                l4[:], hpart[(p, 0)][:, :, 0:128], hpart[(p, 1)][:, :, 0:128],
                op=op,
            )
            dst = mx if p == 0 else mn
            nc.vector.tensor_reduce(dst[:], l4[:], axis=AxX, op=op)

        # Epilogue on [128, RT]:
        # dist_ap = sqrt(relu(8*mx - 6)); exact 0 when a row has no positives.
        u = stat.tile([128, RT], f32, tag="u")
        nc.scalar.activation(u[:], mx[:], Act.Relu, bias=b_m6[:], scale=8.0)
        dap = stat.tile([128, RT], f32, tag="dap")
        nc.scalar.activation(dap[:], u[:], Act.Sqrt)
        # dist_an^2 = max(8*mn + 2, eps) = relu(8*mn + 2 - eps) + eps;
        # >= 8 when a row has no negatives (then dan = 1).
        v = stat.tile([128, RT], f32, tag="v")
        nc.scalar.activation(v[:], mn[:], Act.Relu, bias=b_2me[:], scale=8.0)
        sv = stat.tile([128, RT], f32, tag="sv")
        nc.scalar.activation(sv[:], v[:], Act.Sqrt, bias=b_eps[:])
        e = stat.tile([128, RT], f32, tag="e")
        nc.vector.tensor_scalar(e[:], v[:], 6.0, None, op0=Alu.is_gt)
        # dan = sv + e*(1 - sv);  df = dap - dan
        t1 = stat.tile([128, RT], f32, tag="t1")
        nc.vector.tensor_tensor(t1[:], e[:], sv[:], op=Alu.mult)
        t2 = stat.tile([128, RT], f32, tag="t2")
        nc.vector.tensor_tensor(t2[:], e[:], t1[:], op=Alu.subtract)
        dan = stat.tile([128, RT], f32, tag="dan")
        nc.vector.tensor_tensor(dan[:], sv[:], t2[:], op=Alu.add)
        df = stat.tile([128, RT], f32, tag="df")
        nc.vector.tensor_tensor(df[:], dap[:], dan[:], op=Alu.subtract)
        lrow = stat.tile([128, RT], f32, tag="lrow")
        nc.vector.tensor_scalar(
            lrow[:], df[:], MARGIN, 0.0, op0=Alu.add, op1=Alu.max
        )

        # Ship per-row losses; the host sums 8 x [128, RT] partials.
        nc.sync.dma_start(out[:], lrow[:])

    nc.compile()
    return nc


def _get_nc():
    if "nc" not in _CACHE:
        _CACHE["nc"] = _build()
    return _CACHE["nc"]


def make_in_maps(inputs: np.ndarray, targets: np.ndarray):
    x16 = np.asarray(inputs, dtype=np.float16)
    tf = targets.astype(np.float16)
    in_maps = []
    for r in range(NCORES):
        rows = slice(r * RPC, (r + 1) * RPC)
        opp = slice(HALF, N) if r * RPC < HALF else slice(0, HALF)
        xall = np.concatenate([x16[rows], x16[opp]], axis=0)  # [2560, 256]
        in_maps.append(
            {
                # tile t = rows 128t..128t+127; partition p = row 128t+p
                "xin": np.ascontiguousarray(
                    xall.reshape(NT, 128, D).transpose(1, 0, 2).reshape(128, NT * D)
                ),
                "tb": np.ascontiguousarray(
                    targets[rows].astype(np.float32).reshape(RT, 128).T
                ),
                "to": tf[opp].reshape(1, HALF),
            }
        )
    return in_maps


def kernel(inputs: np.ndarray, targets: np.ndarray) -> np.ndarray:
    from concourse.bass_utils import run_bass_kernel_spmd

    nc = _get_nc()
    in_maps = make_in_maps(inputs, targets)
    res = run_bass_kernel_spmd(nc, in_maps, list(range(NCORES)))
    total = sum(float(res.results[i]["out"].sum()) for i in range(NCORES))
    return np.float32(total / N)


# revision 36
# speedup vs baseline: 1.0504x; 1.0504x over previous
"""Trainium2 Bass kernel for a hard-triplet margin-ranking loss.

Sharding: data-parallel over anchor rows, 8 cores x 512 anchors. Rows in the
first half of the batch mine over columns [2048:4096], second half over
[0:2048]; each core computes its 512x2048 slice of the distance matrix.

Host prep (free, outside HW time): cast features to fp16 and tile them
[128, 20*256] (tile t = rows 128t..128t+127). fp16 halves DMA bytes
(memory-bound regime) and unlocks full-rate PE matmuls (1cyc/row vs 4 for
fp32) plus 2x/4x DVE perf modes.

Per core, per group g of 4 row tiles (pipelined):
  1. DMA the group's natural-layout fp16 x.
  2. Row norms: ACT Square+accum per [128,256] tile; per-group stat chain
     inv = 1/(sqrt(sq)+eps) on [128,4] (anchor group scaled by -0.25 so the
     matmul yields pm = -0.25*<xn_i,xn_j>, dist^2 = 2 + 8*pm).
  3. Normalize: DVE tensor_scalar (4x mode, fp16) into xnall.
  4. One XBAR DMA-transpose [128,1024] -> xT2[:, 8g:8g+8, :]; block e=2t+c
     holds (tile t, dim-chunk c) columns. No PE transposes, no PSUM evac.
Then:
  5. Main matmul: stationary operand (anchor block) reused across 4 moving
     chunks -> 8 Ldweights total; fp16, K=256 via 2 PSUM-accumulated chunks;
     pm [128,2048] f32, double buffered.
  6. w = pm + (t_o == t_b): mask built per row tile (DVE tensor_scalar
     is_equal, 4x); added while evacuating PSUM (engine per CFG knob).
     Matched cols land in [0.75,1.25], unmatched in [-0.25,0.25].
  7. Row max/min via pairwise tensor_tensor trees (2x fp16) on DVE.
  8. Epilogue as in the reference; ones-matmul row-sum; host sums 8
     partials / 4096.
"""

import numpy as np

N, D = 4096, 256
HALF = N // 2
NCORES = 8
RPC = N // NCORES   # 512 anchor rows per core
RT = RPC // 128     # 4 anchor row tiles
OT = HALF // 128    # 16 opposite-half tiles
NT = RT + OT        # 20 input tiles
NG = NT // 4        # 5 groups of 4 tiles
MARGIN = 0.3
EPS = 1e-6

# Engine tuning knobs.
#   wadd[r] in {"dve", "act"}: how pm+mask leaves PSUM for row tile r.
CFG = {
    "wadd2": [["dve", "act", "act", "act"], ["dve", "act", "act", "act"]],
    "sq_dve": set(),
}

_CACHE = {}


def _build():
    from contextlib import ExitStack

    import concourse.bacc as bacc
    import concourse.bass as bass
    import concourse.tile as tile
    from concourse import mybir

    def gsl2(g):
        return slice(g * 4 * D, (g + 1) * 4 * D)

    f32 = mybir.dt.float32
    f16 = mybir.dt.float16
    Alu = mybir.AluOpType
    Act = mybir.ActivationFunctionType
    AxX = mybir.AxisListType.X
    ts = bass.ts

    nc = bacc.Bacc(
        "TRN2",
        target_bir_lowering=False,
        debug=False,
        enable_asserts=True,
        num_devices=NCORES,
    )
    xin = nc.dram_tensor("xin", [128, NT * D], f16, kind="ExternalInput").ap()
    tb = nc.dram_tensor("tb", [128, RT], f32, kind="ExternalInput").ap()
    to = nc.dram_tensor("to", [1, HALF], f16, kind="ExternalInput").ap()
    out = nc.dram_tensor("out", [128, RT], f32, kind="ExternalOutput").ap()

    with tile.TileContext(nc) as tc, ExitStack() as ctx:
        const = ctx.enter_context(tc.tile_pool(name="const", bufs=1))
        xpool = ctx.enter_context(tc.tile_pool(name="xpool", bufs=1))
        stat = ctx.enter_context(tc.tile_pool(name="stat", bufs=1))
        scr = ctx.enter_context(tc.tile_pool(name="scr", bufs=4))
        wmask = ctx.enter_context(tc.tile_pool(name="wmask", bufs=2))
        wpool = ctx.enter_context(tc.tile_pool(name="wpool", bufs=1))
        tree = ctx.enter_context(tc.tile_pool(name="tree", bufs=1))
        psm = ctx.enter_context(tc.tile_pool(name="psm", bufs=3, space="PSUM"))
        psn = ctx.enter_context(tc.tile_pool(name="psn", bufs=2, space="PSUM"))

        ones = const.tile([128, 1], f32, tag="ones")
        nc.vector.memset(ones[:], 1.0)
        # Pin the ACT function table (sqrt_and_others: Square/Sqrt/Copy/Relu)
        # once, while DMAs are in flight, to avoid a mid-kernel reload.
        warm = const.tile([1, 1], f32, tag="warm")
        nc.scalar.activation(warm[:], ones[0:1, :], Act.Abs_reciprocal_sqrt)
        b_m6 = const.tile([128, 1], f32, tag="b_m6")
        nc.vector.memset(b_m6[:], -6.0)
        b_2me = const.tile([128, 1], f32, tag="b_2me")
        nc.vector.memset(b_2me[:], 2.0 - EPS)
        b_eps = const.tile([128, 1], f32, tag="b_eps")
        nc.vector.memset(b_eps[:], EPS)
        b_tiny = const.tile([128, 1], f32, tag="b_tiny")
        nc.vector.memset(b_tiny[:], 1e-12)


        # Phase-1, transposed space: XBAR-transpose raw fp16 straight from
        # DRAM (one DMA per 4-tile group, no natural-layout copy), square it,
        # partition-sum via a PE ones-matmul (every output partition gets the
        # row norm^2 -> no layout shuffle), sqrt (ACT) + reciprocal (DVE),
        # then scale columns (rows of x) with 2 strided tensor_tensor ops per
        # group. 1/(||x||+eps) ~= 1/||x|| to 6e-8 relative, far below fp16
        # noise, so eps is dropped.
        xT2r = xpool.tile([128, 2 * NT, 128], f16, tag="xT2r")
        x2T = xpool.tile([128, 2 * NT, 128], f16, tag="x2T")
        xT2 = xpool.tile([128, 2 * NT, 128], f16, tag="xT2")
        invw = xpool.tile([128, NT * 128], f16, tag="invw")
        ones16 = const.tile([128, 128], f16, tag="ones16")
        nc.vector.memset(ones16[:], 1.0)

        for g in range(NG):
            nc.sync.dma_start(
                xT2r[:, 8 * g : 8 * (g + 1), :], xin[:, gsl2(g)], transpose=True
            )
        # Targets after the transposes in the SP queue (needed much later).
        to_row = const.tile([1, HALF], f16, tag="to_row")
        nc.sync.dma_start(to_row[:], to[:])
        tob = const.tile([128, HALF], f16, tag="tob")
        nc.gpsimd.partition_broadcast(tob[:], to_row[:])
        tbt = const.tile([128, RT], f32, tag="tbt")
        nc.sync.dma_start(tbt[:], tb[:])

        xrc = xT2r[:].rearrange("p (t c) r -> p c t r", c=2)
        x2c = x2T[:].rearrange("p (t c) r -> p c t r", c=2)
        xfc = xT2[:].rearrange("p (t c) r -> p c t r", c=2)
        ivc = invw[:].rearrange("p (t r) -> p t r", r=128)

        for g in range(NG):
            gs8 = slice(8 * g, 8 * (g + 1))
            if g in CFG["sq_dve"]:
                nc.vector.tensor_tensor(
                    x2T[:, gs8, :], xT2r[:, gs8, :], xT2r[:, gs8, :], op=Alu.mult
                )
            else:
                nc.scalar.activation(x2T[:, gs8, :], xT2r[:, gs8, :], Act.Square)
            nq = psn.tile([128, 512], f32, tag="nq")
            for c in range(2):
                nc.tensor.matmul(
                    nq[:],
                    lhsT=ones16[:],
                    rhs=x2c[:, c, 4 * g : 4 * (g + 1), :],
                    start=(c == 0),
                    stop=(c == 1),
                )
            iw = invw[:, 512 * g : 512 * (g + 1)]
            nc.scalar.activation(iw, nq[:], Act.Abs_reciprocal_sqrt)
            if g == 0:
                nc.vector.tensor_scalar_mul(iw, iw, -0.25)
            for c in range(2):
                nc.vector.tensor_tensor(
                    xfc[:, c, 4 * g : 4 * (g + 1), :],
                    xrc[:, c, 4 * g : 4 * (g + 1), :],
                    ivc[:, 4 * g : 4 * (g + 1), :],
                    op=Alu.mult,
                )

        wall = wpool.tile([128, RT, HALF], f16, tag="wall")
        for r in range(RT):
            nc.vector.tensor_scalar(wall[:, r, :], tob[:], tbt[:, r : r + 1],
                                    None, op0=Alu.is_equal)

        # Chunk view: xTc[p, c, t, r] = dim 128c+p of row 128t+r.
        xTc = xT2[:].rearrange("p (t c) r -> p c t r", c=2)

        # Main matmul in two half-rounds: half hp covers distance columns
        # [1024*hp, 1024*(hp+1)) and needs only opposite groups 1+2hp, 2+2hp,
        # so round 0 starts before the last transposes land. PSUM: 4 live
        # [128,1024] halves. Evacuation: ACT copy -> fp16, then a gpsimd
        # SWDGE DMA accumulates it onto the mask in w (DVE untouched).
        mx = stat.tile([128, RT], f16, tag="mx")
        mn = stat.tile([128, RT], f16, tag="mn")
        hpart = {}
        for p in range(2):
            for h in range(2):
                hp_t = tree.tile([128, RT, 512], f16, tag=f"hp_{p}_{h}")
                hpart[(p, h)] = hp_t
        for hp in range(2):
            for r in range(RT):
                pm = psm.tile([128, 1024], f32, tag="pm")
                for c in range(2):
                    for k in range(2):
                        hk = 2 * hp + k
                        nc.tensor.matmul(
                            pm[:, ts(k, 512)],
                            lhsT=xTc[:, c, r, :],
                            rhs=xTc[:, c, 4 + 4 * hk : 8 + 4 * hk, :],
                            start=(c == 0),
                            stop=(c == 1),
                        )
                wh = wall[:, r, 1024 * hp : 1024 * (hp + 1)]
                if CFG["wadd2"][hp][r] == "dve":
                    nc.vector.tensor_tensor(wh, pm[:], wh, op=Alu.add)
                else:
                    tmp = scr.tile([128, 1024], f16, tag="evac")
                    nc.scalar.copy(tmp[:], pm[:])
                    nc.gpsimd.dma_start(wh, tmp[:], accum_op=Alu.add)
            # Pair-batched per-half trees: set s covers row tiles 2s,2s+1 so
            # reduction starts after two adds instead of four.
            for s in range(2):
                whv = wall[:, 2 * s : 2 * s + 2, 1024 * hp : 1024 * (hp + 1)]
                for p, op in ((0, Alu.max), (1, Alu.min)):
                    l2 = hpart[(p, hp)][:, 2 * s : 2 * s + 2, :]
                    nc.vector.tensor_tensor(
                        l2, whv[:, :, 0:512], whv[:, :, 512:1024], op=op
                    )
                    nc.vector.tensor_tensor(
                        l2[:, :, 0:256], l2[:, :, 0:256], l2[:, :, 256:512], op=op
                    )
                    nc.vector.tensor_tensor(
                        l2[:, :, 0:128], l2[:, :, 0:128], l2[:, :, 128:256], op=op
                    )
        for p, op in ((0, Alu.max), (1, Alu.min)):
            l4 = tree.tile([128, RT, 128], f16, tag=f"l4_{p}_f")
            nc.vector.tensor_tensor(
                l4[:], hpart[(p, 0)][:, :, 0:128], hpart[(p, 1)][:, :, 0:128],
                op=op,
            )
            dst = mx if p == 0 else mn
            nc.vector.tensor_reduce(dst[:], l4[:], axis=AxX, op=op)

        # Epilogue on [128, RT]:
        # dist_ap = sqrt(relu(8*mx - 6)); exact 0 when a row has no positives.
        u = stat.tile([128, RT], f32, tag="u")
        nc.scalar.activation(u[:], mx[:], Act.Relu, bias=b_m6[:], scale=8.0)
        rap = stat.tile([128, RT], f32, tag="rap")
        nc.scalar.activation(rap[:], u[:], Act.Abs_reciprocal_sqrt, bias=b_tiny[:])
        dap = stat.tile([128, RT], f32, tag="dap")
        nc.vector.tensor_tensor(dap[:], u[:], rap[:], op=Alu.mult)
        # dist_an^2 = max(8*mn + 2, eps) = relu(8*mn + 2 - eps) + eps;
        # >= 8 when a row has no negatives (then dan = 1).
        v = stat.tile([128, RT], f32, tag="v")
        nc.scalar.activation(v[:], mn[:], Act.Relu, bias=b_2me[:], scale=8.0)
        rv = stat.tile([128, RT], f32, tag="rv")
        nc.scalar.activation(rv[:], v[:], Act.Abs_reciprocal_sqrt, bias=b_eps[:])
        vv = stat.tile([128, RT], f32, tag="vv")
        nc.vector.tensor_scalar_add(vv[:], v[:], EPS)
        sv = stat.tile([128, RT], f32, tag="sv")
        nc.vector.tensor_tensor(sv[:], vv[:], rv[:], op=Alu.mult)
        e = stat.tile([128, RT], f32, tag="e")
        nc.vector.tensor_scalar(e[:], v[:], 6.0, None, op0=Alu.is_gt)
        # dan = sv + e*(1 - sv);  df = dap - dan
        t1 = stat.tile([128, RT], f32, tag="t1")
        nc.vector.tensor_tensor(t1[:], e[:], sv[:], op=Alu.mult)
        t2 = stat.tile([128, RT], f32, tag="t2")
        nc.vector.tensor_tensor(t2[:], e[:], t1[:], op=Alu.subtract)
        dan = stat.tile([128, RT], f32, tag="dan")
        nc.vector.tensor_tensor(dan[:], sv[:], t2[:], op=Alu.add)
        df = stat.tile([128, RT], f32, tag="df")
        nc.vector.tensor_tensor(df[:], dap[:], dan[:], op=Alu.subtract)
        lrow = stat.tile([128, RT], f32, tag="lrow")
        nc.vector.tensor_scalar(
            lrow[:], df[:], MARGIN, 0.0, op0=Alu.add, op1=Alu.max
        )

        # Ship per-row losses; the host sums 8 x [128, RT] partials.
        nc.sync.dma_start(out[:], lrow[:])

    nc.compile()
    return nc


def _get_nc():
    if "nc" not in _CACHE:
        _CACHE["nc"] = _build()
    return _CACHE["nc"]


def make_in_maps(inputs: np.ndarray, targets: np.ndarray):
    x16 = np.asarray(inputs, dtype=np.float16)
    tf = targets.astype(np.float16)
    in_maps = []
    for r in range(NCORES):
        rows = slice(r * RPC, (r + 1) * RPC)
        opp = slice(HALF, N) if r * RPC < HALF else slice(0, HALF)
        xall = np.concatenate([x16[rows], x16[opp]], axis=0)  # [2560, 256]
        in_maps.append(
            {
                # tile t = rows 128t..128t+127; partition p = row 128t+p
                "xin": np.ascontiguousarray(
                    xall.reshape(NT, 128, D).transpose(1, 0, 2).reshape(128, NT * D)
                ),
                "tb": np.ascontiguousarray(
                    targets[rows].astype(np.float32).reshape(RT, 128).T
                ),
                "to": tf[opp].reshape(1, HALF),
            }
        )
    return in_maps


def kernel(inputs: np.ndarray, targets: np.ndarray) -> np.ndarray:
    from concourse.bass_utils import run_bass_kernel_spmd

    nc = _get_nc()
    in_maps = make_in_maps(inputs, targets)
    res = run_bass_kernel_spmd(nc, in_maps, list(range(NCORES)))
    total = sum(float(res.results[i]["out"].sum()) for i in range(NCORES))
    return np.float32(total / N)


# revision 37
# speedup vs baseline: 1.0767x; 1.0251x over previous
"""Trainium2 Bass kernel for a hard-triplet margin-ranking loss.

Sharding: data-parallel over anchor rows, 8 cores x 512 anchors. Rows in the
first half of the batch mine over columns [2048:4096], second half over
[0:2048]; each core computes its 512x2048 slice of the distance matrix.

Host prep (free, outside HW time): cast features to fp16 and tile them
[128, 20*256] (tile t = rows 128t..128t+127). fp16 halves DMA bytes
(memory-bound regime) and unlocks full-rate PE matmuls (1cyc/row vs 4 for
fp32) plus 2x/4x DVE perf modes.

Per core, per group g of 4 row tiles (pipelined):
  1. DMA the group's natural-layout fp16 x.
  2. Row norms: ACT Square+accum per [128,256] tile; per-group stat chain
     inv = 1/(sqrt(sq)+eps) on [128,4] (anchor group scaled by -0.25 so the
     matmul yields pm = -0.25*<xn_i,xn_j>, dist^2 = 2 + 8*pm).
  3. Normalize: DVE tensor_scalar (4x mode, fp16) into xnall.
  4. One XBAR DMA-transpose [128,1024] -> xT2[:, 8g:8g+8, :]; block e=2t+c
     holds (tile t, dim-chunk c) columns. No PE transposes, no PSUM evac.
Then:
  5. Main matmul: stationary operand (anchor block) reused across 4 moving
     chunks -> 8 Ldweights total; fp16, K=256 via 2 PSUM-accumulated chunks;
     pm [128,2048] f32, double buffered.
  6. w = pm + (t_o == t_b): mask built per row tile (DVE tensor_scalar
     is_equal, 4x); added while evacuating PSUM (engine per CFG knob).
     Matched cols land in [0.75,1.25], unmatched in [-0.25,0.25].
  7. Row max/min via pairwise tensor_tensor trees (2x fp16) on DVE.
  8. Epilogue as in the reference; ones-matmul row-sum; host sums 8
     partials / 4096.
"""

import numpy as np

N, D = 4096, 256
HALF = N // 2
NCORES = 8
RPC = N // NCORES   # 512 anchor rows per core
RT = RPC // 128     # 4 anchor row tiles
OT = HALF // 128    # 16 opposite-half tiles
NT = RT + OT        # 20 input tiles
NG = NT // 4        # 5 groups of 4 tiles
MARGIN = 0.3
EPS = 1e-6

# Engine tuning knobs.
#   wadd[r] in {"dve", "act"}: how pm+mask leaves PSUM for row tile r.
CFG = {
    "wadd2": [["act", "act", "act", "act"], ["dve", "act", "act", "act"]],
    "sq_dve": set(),
}

_CACHE = {}


def _build():
    from contextlib import ExitStack

    import concourse.bacc as bacc
    import concourse.bass as bass
    import concourse.tile as tile
    from concourse import mybir

    def gsl2(g):
        return slice(g * 4 * D, (g + 1) * 4 * D)

    f32 = mybir.dt.float32
    f16 = mybir.dt.float16
    Alu = mybir.AluOpType
    Act = mybir.ActivationFunctionType
    AxX = mybir.AxisListType.X
    ts = bass.ts

    nc = bacc.Bacc(
        "TRN2",
        target_bir_lowering=False,
        debug=False,
        enable_asserts=True,
        num_devices=NCORES,
    )
    xin = nc.dram_tensor("xin", [128, NT * D], f16, kind="ExternalInput").ap()
    tb = nc.dram_tensor("tb", [128, RT], f32, kind="ExternalInput").ap()
    to = nc.dram_tensor("to", [1, HALF], f16, kind="ExternalInput").ap()
    out = nc.dram_tensor("out", [128, RT], f32, kind="ExternalOutput").ap()

    with tile.TileContext(nc) as tc, ExitStack() as ctx:
        const = ctx.enter_context(tc.tile_pool(name="const", bufs=1))
        xpool = ctx.enter_context(tc.tile_pool(name="xpool", bufs=1))
        stat = ctx.enter_context(tc.tile_pool(name="stat", bufs=1))
        scr = ctx.enter_context(tc.tile_pool(name="scr", bufs=4))
        wmask = ctx.enter_context(tc.tile_pool(name="wmask", bufs=2))
        wpool = ctx.enter_context(tc.tile_pool(name="wpool", bufs=1))
        tree = ctx.enter_context(tc.tile_pool(name="tree", bufs=1))
        psm = ctx.enter_context(tc.tile_pool(name="psm", bufs=3, space="PSUM"))
        psn = ctx.enter_context(tc.tile_pool(name="psn", bufs=2, space="PSUM"))

        ones = const.tile([128, 1], f32, tag="ones")
        nc.vector.memset(ones[:], 1.0)
        # Pin the ACT function table (sqrt_and_others: Square/Sqrt/Copy/Relu)
        # once, while DMAs are in flight, to avoid a mid-kernel reload.
        warm = const.tile([1, 1], f32, tag="warm")
        nc.scalar.activation(warm[:], ones[0:1, :], Act.Abs_reciprocal_sqrt)
        b_m6 = const.tile([128, 1], f32, tag="b_m6")
        nc.vector.memset(b_m6[:], -6.0)
        b_2me = const.tile([128, 1], f32, tag="b_2me")
        nc.vector.memset(b_2me[:], 2.0 - EPS)
        b_eps = const.tile([128, 1], f32, tag="b_eps")
        nc.vector.memset(b_eps[:], EPS)
        b_tiny = const.tile([128, 1], f32, tag="b_tiny")
        nc.vector.memset(b_tiny[:], 1e-12)


        # Phase-1, transposed space: XBAR-transpose raw fp16 straight from
        # DRAM (one DMA per 4-tile group, no natural-layout copy), square it,
        # partition-sum via a PE ones-matmul (every output partition gets the
        # row norm^2 -> no layout shuffle), sqrt (ACT) + reciprocal (DVE),
        # then scale columns (rows of x) with 2 strided tensor_tensor ops per
        # group. 1/(||x||+eps) ~= 1/||x|| to 6e-8 relative, far below fp16
        # noise, so eps is dropped.
        xT2r = xpool.tile([128, 2 * NT, 128], f16, tag="xT2r")
        x2T = xpool.tile([128, 2 * NT, 128], f16, tag="x2T")
        xT2 = xpool.tile([128, 2 * NT, 128], f16, tag="xT2")
        invw = xpool.tile([128, NT * 128], f16, tag="invw")
        ones16 = const.tile([128, 128], f16, tag="ones16")
        nc.vector.memset(ones16[:], 1.0)

        for g in range(NG):
            nc.sync.dma_start(
                xT2r[:, 8 * g : 8 * (g + 1), :], xin[:, gsl2(g)], transpose=True
            )
        # Targets after the transposes in the SP queue (needed much later).
        to_row = const.tile([1, HALF], f16, tag="to_row")
        nc.sync.dma_start(to_row[:], to[:])
        tob = const.tile([128, HALF], f16, tag="tob")
        nc.gpsimd.partition_broadcast(tob[:], to_row[:])
        tbt = const.tile([128, RT], f32, tag="tbt")
        nc.sync.dma_start(tbt[:], tb[:])

        xrc = xT2r[:].rearrange("p (t c) r -> p c t r", c=2)
        x2c = x2T[:].rearrange("p (t c) r -> p c t r", c=2)
        xfc = xT2[:].rearrange("p (t c) r -> p c t r", c=2)
        ivc = invw[:].rearrange("p (t r) -> p t r", r=128)

        for g in range(NG):
            gs8 = slice(8 * g, 8 * (g + 1))
            if g in CFG["sq_dve"]:
                nc.vector.tensor_tensor(
                    x2T[:, gs8, :], xT2r[:, gs8, :], xT2r[:, gs8, :], op=Alu.mult
                )
            else:
                nc.scalar.activation(x2T[:, gs8, :], xT2r[:, gs8, :], Act.Square)
            nq = psn.tile([128, 512], f32, tag="nq")
            for c in range(2):
                nc.tensor.matmul(
                    nq[:],
                    lhsT=ones16[:],
                    rhs=x2c[:, c, 4 * g : 4 * (g + 1), :],
                    start=(c == 0),
                    stop=(c == 1),
                )
            iw = invw[:, 512 * g : 512 * (g + 1)]
            nc.scalar.activation(iw, nq[:], Act.Abs_reciprocal_sqrt)
            if g == 0:
                nc.vector.tensor_scalar_mul(iw, iw, -0.25)
            for c in range(2):
                nc.vector.tensor_tensor(
                    xfc[:, c, 4 * g : 4 * (g + 1), :],
                    xrc[:, c, 4 * g : 4 * (g + 1), :],
                    ivc[:, 4 * g : 4 * (g + 1), :],
                    op=Alu.mult,
                )

        wall = wpool.tile([128, RT, HALF], f16, tag="wall")
        for r in range(RT):
            nc.vector.tensor_scalar(wall[:, r, :], tob[:], tbt[:, r : r + 1],
                                    None, op0=Alu.is_equal)

        # Chunk view: xTc[p, c, t, r] = dim 128c+p of row 128t+r.
        xTc = xT2[:].rearrange("p (t c) r -> p c t r", c=2)

        # Main matmul in two half-rounds: half hp covers distance columns
        # [1024*hp, 1024*(hp+1)) and needs only opposite groups 1+2hp, 2+2hp,
        # so round 0 starts before the last transposes land. PSUM: 4 live
        # [128,1024] halves. Evacuation: ACT copy -> fp16, then a gpsimd
        # SWDGE DMA accumulates it onto the mask in w (DVE untouched).
        mx = stat.tile([128, RT], f16, tag="mx")
        mn = stat.tile([128, RT], f16, tag="mn")
        hpart = {}
        for p in range(2):
            for h in range(2):
                hp_t = tree.tile([128, RT, 512], f16, tag=f"hp_{p}_{h}")
                hpart[(p, h)] = hp_t
        for hp in range(2):
            for r in range(RT):
                pm = psm.tile([128, 1024], f32, tag="pm")
                for c in range(2):
                    for k in range(2):
                        hk = 2 * hp + k
                        nc.tensor.matmul(
                            pm[:, ts(k, 512)],
                            lhsT=xTc[:, c, r, :],
                            rhs=xTc[:, c, 4 + 4 * hk : 8 + 4 * hk, :],
                            start=(c == 0),
                            stop=(c == 1),
                        )
                wh = wall[:, r, 1024 * hp : 1024 * (hp + 1)]
                if CFG["wadd2"][hp][r] == "dve":
                    nc.vector.tensor_tensor(wh, pm[:], wh, op=Alu.add)
                else:
                    tmp = scr.tile([128, 1024], f16, tag="evac")
                    nc.scalar.copy(tmp[:], pm[:])
                    nc.gpsimd.dma_start(wh, tmp[:], accum_op=Alu.add)
            # Pair-batched per-half trees: set s covers row tiles 2s,2s+1 so
            # reduction starts after two adds instead of four.
            for s in range(2):
                whv = wall[:, 2 * s : 2 * s + 2, 1024 * hp : 1024 * (hp + 1)]
                for p, op in ((0, Alu.max), (1, Alu.min)):
                    l2 = hpart[(p, hp)][:, 2 * s : 2 * s + 2, :]
                    nc.vector.tensor_tensor(
                        l2, whv[:, :, 0:512], whv[:, :, 512:1024], op=op
                    )
                    nc.vector.tensor_tensor(
                        l2[:, :, 0:256], l2[:, :, 0:256], l2[:, :, 256:512], op=op
                    )
                    nc.vector.tensor_tensor(
                        l2[:, :, 0:128], l2[:, :, 0:128], l2[:, :, 128:256], op=op
                    )
        for p, op in ((0, Alu.max), (1, Alu.min)):
            l4 = tree.tile([128, RT, 128], f16, tag=f"l4_{p}_f")
            nc.vector.tensor_tensor(
                l4[:], hpart[(p, 0)][:, :, 0:128], hpart[(p, 1)][:, :, 0:128],
                op=op,
            )
            dst = mx if p == 0 else mn
            nc.vector.tensor_reduce(dst[:], l4[:], axis=AxX, op=op)

        # Epilogue on [128, RT]:
        # dist_ap = sqrt(relu(8*mx - 6)); exact 0 when a row has no positives.
        u = stat.tile([128, RT], f32, tag="u")
        nc.scalar.activation(u[:], mx[:], Act.Relu, bias=b_m6[:], scale=8.0)
        rap = stat.tile([128, RT], f32, tag="rap")
        nc.scalar.activation(rap[:], u[:], Act.Abs_reciprocal_sqrt, bias=b_tiny[:])
        dap = stat.tile([128, RT], f32, tag="dap")
        nc.vector.tensor_tensor(dap[:], u[:], rap[:], op=Alu.mult)
        # dist_an^2 = max(8*mn + 2, eps) = relu(8*mn + 2 - eps) + eps;
        # >= 8 when a row has no negatives (then dan = 1).
        v = stat.tile([128, RT], f32, tag="v")
        nc.scalar.activation(v[:], mn[:], Act.Relu, bias=b_2me[:], scale=8.0)
        rv = stat.tile([128, RT], f32, tag="rv")
        nc.scalar.activation(rv[:], v[:], Act.Abs_reciprocal_sqrt, bias=b_eps[:])
        vv = stat.tile([128, RT], f32, tag="vv")
        nc.vector.tensor_scalar_add(vv[:], v[:], EPS)
        sv = stat.tile([128, RT], f32, tag="sv")
        nc.vector.tensor_tensor(sv[:], vv[:], rv[:], op=Alu.mult)
        e = stat.tile([128, RT], f32, tag="e")
        nc.vector.tensor_scalar(e[:], v[:], 6.0, None, op0=Alu.is_gt)
        # dan = sv + e*(1 - sv);  df = dap - dan
        t1 = stat.tile([128, RT], f32, tag="t1")
        nc.vector.tensor_tensor(t1[:], e[:], sv[:], op=Alu.mult)
        t2 = stat.tile([128, RT], f32, tag="t2")
        nc.vector.tensor_tensor(t2[:], e[:], t1[:], op=Alu.subtract)
        dan = stat.tile([128, RT], f32, tag="dan")
        nc.vector.tensor_tensor(dan[:], sv[:], t2[:], op=Alu.add)
        df = stat.tile([128, RT], f32, tag="df")
        nc.vector.tensor_tensor(df[:], dap[:], dan[:], op=Alu.subtract)
        lrow = stat.tile([128, RT], f32, tag="lrow")
        nc.vector.tensor_scalar(
            lrow[:], df[:], MARGIN, 0.0, op0=Alu.add, op1=Alu.max
        )

        # Ship per-row losses; the host sums 8 x [128, RT] partials.
        nc.sync.dma_start(out[:], lrow[:])

    nc.compile()
    return nc


def _get_nc():
    if "nc" not in _CACHE:
        _CACHE["nc"] = _build()
    return _CACHE["nc"]


def make_in_maps(inputs: np.ndarray, targets: np.ndarray):
    x16 = np.asarray(inputs, dtype=np.float16)
    tf = targets.astype(np.float16)
    in_maps = []
    for r in range(NCORES):
        rows = slice(r * RPC, (r + 1) * RPC)
        opp = slice(HALF, N) if r * RPC < HALF else slice(0, HALF)
        xall = np.concatenate([x16[rows], x16[opp]], axis=0)  # [2560, 256]
        in_maps.append(
            {
                # tile t = rows 128t..128t+127; partition p = row 128t+p
                "xin": np.ascontiguousarray(
                    xall.reshape(NT, 128, D).transpose(1, 0, 2).reshape(128, NT * D)
                ),
                "tb": np.ascontiguousarray(
                    targets[rows].astype(np.float32).reshape(RT, 128).T
                ),
                "to": tf[opp].reshape(1, HALF),
            }
        )
    return in_maps


def kernel(inputs: np.ndarray, targets: np.ndarray) -> np.ndarray:
    from concourse.bass_utils import run_bass_kernel_spmd

    nc = _get_nc()
    in_maps = make_in_maps(inputs, targets)
    res = run_bass_kernel_spmd(nc, in_maps, list(range(NCORES)))
    total = sum(float(res.results[i]["out"].sum()) for i in range(NCORES))
    return np.float32(total / N)
